# revision 70
# baseline (speedup 1.0000x reference)
"""Trainium2 Bass kernel for a BERT-style self-attention block (B=2, S=4096,
H=768, NH=12) sharded over 8 NeuronCores.

Sharding: data-parallel over batch (2) x query-block parallel (4) = 8 cores.
Each core computes K/V for the full sequence of its batch (replicated within
the 4-core group) and a disjoint 1024-query slice of the output, so no
collectives are needed and the LayerNorm epilogue is fully local.

Per-core dataflow (fp8 DoubleRow matmuls where contraction >= 256):
  Projections: x8 [H,S] fp8, weights fp8 (scaled x16/x32 into e4m3 range),
    K^T/Q^T stored bf16 (ACT Identity conv, per-channel bias, 1/16 & 1/128
    descale), V stored fp8 with exp(mask) folded into its rows (additive
    mask == multiplicative exp(mask) on unnormalized probs) and the v-bias
    routed through softmax into the host-side residual (vb passes softmax
    normalization untouched: ctx = raw + vb).
  Scores: bf16, per head at PE row-offset (h%2)*64, true-scale in PSUM.
    Both 128-key tiles of a pair land in ONE 2-bank PSUM tile (unified
    3-buf "big" pool shared with proj/tail) so each exp op is 1024 wide.
  exp: split across ACT (AF.Exp, fp8 out) and DVE (Schraudolph: one
    tensor_scalar mult+add writing the fp8e4m3 BITS as int8), strict A/V
    alternation per skey-chunk-pair (GpSimd/DMA cannot read PSUM, so only
    these two engines can drain scores; GpSimd instead absorbs all
    SBUF-side work: LN chain, memsets, copies).
  ctx: fp8 DoubleRow over skey-chunk pairs, ones-column = exp(mask)/32 so
    row 64 of the accumulator is the softmax denominator/32; normalize =
    reciprocal_approx_fast + DRAM-bounce broadcast + one DVE mult writing
    ctxT fp8 (x32 scaled).
  Tail: O-proj fp8 DR (PSUM = 1024*ctx@o_w.T), y = po + 1024*(x+o_b+vb@o_w.T)
    (LN is scale-invariant), mean/var fp32, normalize via fused
    affine_mul_reduce (*gamma) + affine_then_add (+beta).
"""

import numpy as np
import ml_dtypes

B, S, H, NH = 2, 4096, 768, 12
HD = H // NH  # 64
NCORES = 8
SQ = S // 4  # 1024 queries per core
LN_EPS = 1e-12

# Schraudolph exp in fp8e4m3 bit-space: bits = round(s*8/ln2 + B8)
EXP_A = 8.0 / np.log(2.0)
EXP_B8 = 56.0 - 0.458

_BUILD_CACHE = {}


def build(S_=S, SQ_=SQ, stage="full", repeat=1, exp_pat="AV",
          spb=3, ptb=8, rdb_=3, xtb=3, conv_pat="VA", norm_gp=1, tail_lag=3,
          rq=False):
    key = (S_, SQ_, stage, repeat, exp_pat, spb, ptb, rdb_, xtb, conv_pat,
           norm_gp, tail_lag, rq)
    if key in _BUILD_CACHE:
        return _BUILD_CACHE[key]

    import concourse.mybir as mybir
    import concourse.tile as tile
    from concourse import bacc

    dt = mybir.dt
    f32, bf16, f8, i8 = dt.float32, dt.bfloat16, dt.float8e4, dt.int8
    AF = mybir.ActivationFunctionType
    OP = mybir.AluOpType
    PM = mybir.MatmulPerfMode

    EC = H // 128            # 6 contraction chunks over H
    OC = H // 128            # 6 output-channel chunks over H
    NSC512 = S_ // 512       # 512-wide s chunks (projection phase)
    NST = S_ // 128          # 128-wide skey tiles
    NST2 = NST // 2          # skey tile pairs (ctx DoubleRow)
    QT = min(512, SQ_)       # q tile for attention
    NQT = SQ_ // QT
    QCH = min(512, SQ_)      # q chunk for Q projection
    NQCH = SQ_ // QCH
    NSTQ = SQ_ // 128        # output s tiles

    nc = bacc.Bacc("TRN2", target_bir_lowering=False, debug=False)

    xT8 = nc.dram_tensor("xT8", [H, S_], f8, kind="ExternalInput")
    xTq8 = nc.dram_tensor("xTq8", [H, SQ_], f8, kind="ExternalInput")
    wq8 = nc.dram_tensor("wq8", [H, H], f8, kind="ExternalInput")
    wk8 = nc.dram_tensor("wk8", [H, H], f8, kind="ExternalInput")
    wv8 = nc.dram_tensor("wv8", [H, H], f8, kind="ExternalInput")
    wo8 = nc.dram_tensor("wo8", [H, H], f8, kind="ExternalInput")
    qb8 = nc.dram_tensor("qb8", [H], f32, kind="ExternalInput")
    kb = nc.dram_tensor("kb", [H], f32, kind="ExternalInput")
    emsk16 = nc.dram_tensor("emsk16", [S_], f32, kind="ExternalInput")
    emsk32 = nc.dram_tensor("emsk32", [S_], f32, kind="ExternalInput")
    xres = nc.dram_tensor("xres", [SQ_, H], f32, kind="ExternalInput")
    lng = nc.dram_tensor("lng", [H], f32, kind="ExternalInput")
    lnb = nc.dram_tensor("lnb", [H], f32, kind="ExternalInput")
    out = nc.dram_tensor("out", [SQ_, H], f32, kind="ExternalOutput")

    def emit_qkv(nc, P):
        # PSUM->SBUF conversions rotate across DVE/ACT so no single engine
        # gates the projection phase (ACT alone was 95% busy here).  GpSimd
        # cannot touch PSUM (BIR verifier), so it only gets SBUF-side work.
        cstate = {"i": 0}

        def conv(out, psum, scale, bias, af, force=None):
            if force is None:
                e = conv_pat[cstate["i"] % len(conv_pat)]
                cstate["i"] += 1
            else:
                e = force
            if e == "A":
                nc.scalar.activation(
                    out, psum, af,
                    bias=0.0 if bias is None else bias, scale=scale,
                )
            else:
                if bias is None:
                    nc.vector.tensor_scalar(out, psum, scale, None, OP.mult)
                else:
                    nc.vector.tensor_scalar(
                        out, psum, scale, bias, OP.mult, OP.add
                    )

        # Q projection first: small, unblocks attention early.  Weight DMA
        # order: wq is emitted before this (Q-proj dep); wk/wv/wo queue AFTER
        # the first xtq tile so the PE isn't stalled behind 2.4MB of weight
        # loads it doesn't need yet.
        xTq_r = xTq8.rearrange("(c p) s -> p c s", p=128)
        for qc in range(NQCH):
            xtq = P.xtp.tile([128, EC, 512], f8, tag="xt")
            nc.sync.dma_start(xtq[:, :, 0:QCH], xTq_r[:, :, qc * QCH:(qc + 1) * QCH])
            if qc == 0:
                if P.persist_dmas is not None:
                    P.persist_dmas()
                    P.persist_dmas = None
                for wsb, wdr in ((P.wk_sb, wk8), (P.wv_sb, wv8),
                                 (P.wo_sb, wo8)):
                    nc.sync.dma_start(
                        wsb[:], wdr.rearrange("(c p) o -> p c o", p=128)
                    )
                if P.persist_dmas_late is not None:
                    P.persist_dmas_late()
                    P.persist_dmas_late = None
            for oc2 in range(OC // 2):
                pq = P.big.tile([128, 2 * 512], f32, tag="big")
                for half in (0, 1):
                    oc = 2 * oc2 + half
                    for e in range(EC // 2):
                        nc.tensor.matmul(
                            pq[:, half * 512:half * 512 + QCH],
                            P.wq_sb[:, 2 * e:2 * e + 2, oc * 128:(oc + 1) * 128],
                            xtq[:, 2 * e:2 * e + 2, 0:QCH],
                            start=(e == 0), stop=(e == EC // 2 - 1),
                            perf_mode=PM.DoubleRow,
                        )
                for half in (0, 1):
                    oc = 2 * oc2 + half
                    conv(
                        P.qt_sb[:, oc, qc * QCH:(qc + 1) * QCH],
                        pq[:, half * 512:half * 512 + QCH],
                        1.0 / 128.0, P.qb_sb[:, oc:oc + 1], AF.Identity,
                    )
        # K^T and V over the full sequence
        xT_r = xT8.rearrange("(c p) s -> p c s", p=128)
        for sc in range(NSC512):
            xt = P.xtp.tile([128, EC, 512], f8, tag="xt")
            nc.sync.dma_start(xt[:], xT_r[:, :, sc * 512:(sc + 1) * 512])
            for oc2 in range(OC // 2):
                pk = P.big.tile([128, 2 * 512], f32, tag="big")
                for half in (0, 1):
                    oc = 2 * oc2 + half
                    for e in range(EC // 2):
                        nc.tensor.matmul(
                            pk[:, half * 512:(half + 1) * 512],
                            P.wk_sb[:, 2 * e:2 * e + 2, oc * 128:(oc + 1) * 128],
                            xt[:, 2 * e:2 * e + 2, :],
                            start=(e == 0), stop=(e == EC // 2 - 1),
                            perf_mode=PM.DoubleRow,
                        )
                for half in (0, 1):
                    oc = 2 * oc2 + half
                    conv(
                        P.kt_sb[:, oc, sc * 512:(sc + 1) * 512],
                        pk[:, half * 512:(half + 1) * 512],
                        1.0 / 16.0, P.kb_sb[:, oc:oc + 1], AF.Identity,
                    )
            for t4 in range(4):
                st = sc * 4 + t4
                # One 2-bank tile holds both V-proj halves contiguously so a
                # single conv op covers all 768 channels.
                pv = P.big.tile([128, 2 * QT], f32, tag="big")
                for e in range(EC // 2):
                    xs = xt[:, 2 * e:2 * e + 2, t4 * 128:(t4 + 1) * 128]
                    nc.tensor.matmul(
                        pv[:, 0:512], xs, P.wv_sb[:, 2 * e:2 * e + 2, 0:512],
                        start=(e == 0), stop=(e == EC // 2 - 1),
                        perf_mode=PM.DoubleRow,
                    )
                    nc.tensor.matmul(
                        pv[:, 512:H], xs, P.wv_sb[:, 2 * e:2 * e + 2, 512:H],
                        start=(e == 0), stop=(e == EC // 2 - 1),
                        perf_mode=PM.DoubleRow,
                    )
                vview = P.v_sb[:, st].rearrange("p (h d) -> p h d", d=HD + 16)
                conv(
                    vview[:, 0:NH, 0:HD],
                    pv[:, 0:H].rearrange("p (h d) -> p h d", d=HD),
                    P.emsk16_sb[:, st:st + 1], None, AF.Copy,
                )
                nc.gpsimd.tensor_copy(
                    vview[:, 0:NH, HD:HD + 1],
                    P.emsk32_sb[:, st:st + 1].to_broadcast((128, NH, 1)),
                )

    def emit_attention(nc, P):
        do_norm = stage not in ("attn1",)
        for qt in range(NQT):
            q0 = qt * QT
            for h in range(NH):
                # Delayed-tail emission: the previous qt's tail goes into the
                # queues a few heads into THIS qt, so the PE streams scores
                # while the last heads' normalize chains drain, instead of
                # stalling at the qt boundary behind the tail O-proj.
                if stage == "full" and qt > 0 and h == tail_lag:
                    emit_tail(nc, P, list(range((qt - 1) * (QT // 128),
                                                qt * (QT // 128))))
                hp = (h % 2) * 64
                oc = h // 2
                cx = P.psC.tile([128, QT], f32, tag="cx")
                for sc2 in range(NST2):
                    pt = P.ptp.tile([128, 2, QT], f8, tag="pt")
                    # Both 128-key score tiles land in ONE 2-bank PSUM tile so
                    # a single 1024-wide exp op covers the pair (halves the
                    # per-op overhead and the instruction count).
                    sp = P.big.tile([128, 2 * QT], f32, tag="big")
                    for par in (0, 1):
                        sc = 2 * sc2 + par
                        nc.tensor.matmul(
                            sp[:, par * QT:(par + 1) * QT],
                            P.kt_sb[hp:hp + 64, oc, sc * 128:(sc + 1) * 128],
                            P.qt_sb[hp:hp + 64, oc, q0:q0 + QT],
                            start=True, stop=True,
                        )
                    pt_flat = pt.rearrange("p a q -> p (a q)")
                    eng = exp_pat[P.ecnt[0] % len(exp_pat)]
                    P.ecnt[0] += 1
                    if eng == "A":
                        nc.scalar.activation(
                            pt_flat, sp[:], AF.Exp, bias=0.0, scale=1.0
                        )
                    elif eng == "S":
                        # split pair: halves run concurrently on ACT+DVE —
                        # shifts load toward ACT without an A-A seam stall
                        nc.scalar.activation(
                            pt[:, 0, :], sp[:, 0:QT], AF.Exp,
                            bias=0.0, scale=1.0,
                        )
                        nc.vector.tensor_scalar(
                            pt[:, 1, :].bitcast(i8), sp[:, QT:2 * QT],
                            EXP_A, EXP_B8, OP.mult, OP.add,
                        )
                    else:
                        nc.vector.tensor_scalar(
                            pt_flat.bitcast(i8), sp[:],
                            EXP_A, EXP_B8, OP.mult, OP.add,
                        )
                    nc.tensor.matmul(
                        cx[:],
                        P.v_sb[:, 2 * sc2:2 * sc2 + 2,
                               h * (HD + 16):h * (HD + 16) + 128],
                        pt[:],
                        start=(sc2 == 0), stop=(sc2 == NST2 - 1),
                        perf_mode=PM.DoubleRow,
                    )
                if do_norm:
                    if stage == "attn" and qt == 0 and h == 0:
                        den = P.rdp.tile([1, QT], f32, tag="den")
                        nc.vector.tensor_copy(den[:], cx[HD:HD + 1, :])
                        nc.gpsimd.dma_start(out[128:129, 0:QT], den[:])
                    if norm_gp and h % norm_gp == 0:
                        # Drain cx (plus the denominator row) via one ACT
                        # copy so the (SBUF-only, mostly idle) GpSimd can do
                        # the normalize multiply; walrus rejects divide on
                        # Pool, so DVE still does the reciprocal — but on a
                        # [64,8] DRAM-bounced view (DVE cost is free-size
                        # driven: 8 cols, not 512).
                        if rq:
                            # cxs copy alternates A/V; the den bounce rides
                            # the Pool DMA queue (sync stays clear) and the
                            # [64,8] view makes the DVE reciprocal ~free.
                            cxs = P.rdp.tile([HD + 1, QT], f32, tag="cxs")
                            if h % 2 == 0:
                                nc.scalar.activation(
                                    cxs[:], cx[0:HD + 1, :], AF.Copy,
                                    bias=0.0, scale=1.0,
                                )
                            else:
                                nc.vector.tensor_scalar(
                                    cxs[:], cx[0:HD + 1, :], 1.0, None,
                                    OP.mult,
                                )
                            rdd = P.rddr.tile([1, QT], f32, tag="rdd")
                            nc.gpsimd.dma_start(rdd[:], cxs[HD:HD + 1, :])
                            rds = P.rdp.tile([64, QT // 64], f32, tag="rds")
                            nc.gpsimd.dma_start(
                                rds[:],
                                rdd[:].rearrange("o (p c) -> (o p) c", p=64),
                            )
                            rdi = P.rdp.tile([64, QT // 64], f32, tag="rdi")
                            nc.vector.reciprocal(rdi[:], rds[:])
                            rdd2 = P.rddr.tile([64, QT // 64], f32, tag="rdd2")
                            nc.gpsimd.dma_start(rdd2[:], rdi[:])
                            rdb = P.rdp.tile([64, QT], f32, tag="rdb")
                            nc.gpsimd.dma_start(
                                rdb[:],
                                rdd2[:].rearrange("(o p) c -> o (p c)", o=1)
                                .to_broadcast((64, QT)),
                            )
                        else:
                            # recip first: it heads the longest chain
                            # (recip -> DMA -> DMA -> mult)
                            rd = P.rdp.tile([1, QT], f32, tag="rd")
                            nc.vector.reciprocal(rd[:], cx[HD:HD + 1, :])
                            cxs = P.rdp.tile([HD, QT], f32, tag="cxs")
                            nc.scalar.activation(
                                cxs[:], cx[0:HD, :], AF.Copy,
                                bias=0.0, scale=1.0,
                            )
                            rdd = P.rddr.tile([1, QT], f32, tag="rdd")
                            nc.sync.dma_start(rdd[:], rd[:])
                            rdb = P.rdp.tile([64, QT], f32, tag="rdb")
                            nc.sync.dma_start(
                                rdb[:], rdd[:].to_broadcast((64, QT))
                            )
                        nc.gpsimd.tensor_tensor(
                            P.ctxT[hp:hp + 64, oc, q0:q0 + QT],
                            cxs[0:HD, :], rdb[:], OP.mult,
                        )
                    else:
                        rd = P.rdp.tile([1, QT], f32, tag="rd")
                        nc.vector.reciprocal(rd[:], cx[HD:HD + 1, :])
                        rdd = P.rddr.tile([1, QT], f32, tag="rdd")
                        nc.sync.dma_start(rdd[:], rd[:])
                        rdb = P.rdp.tile([64, QT], f32, tag="rdb")
                        nc.sync.dma_start(rdb[:], rdd[:].to_broadcast((64, QT)))
                        nc.vector.tensor_tensor(
                            P.ctxT[hp:hp + 64, oc, q0:q0 + QT],
                            cx[0:HD, :], rdb[:], OP.mult,
                        )
            if stage == "full" and (qt == NQT - 1 or tail_lag == 0):
                emit_tail(nc, P, list(range(qt * (QT // 128), (qt + 1) * (QT // 128))))

    def emit_tail(nc, P, sts):
        ys = {}
        for st in sts:
            # Both O-proj halves in one 2-bank tile: the y-add below becomes
            # a single 768-wide op.
            po = P.big.tile([128, 2 * QT], f32, tag="big")
            for d in range(EC // 2):
                lh = P.ctxT[:, 2 * d:2 * d + 2, st * 128:(st + 1) * 128]
                nc.tensor.matmul(
                    po[:, 0:512], lh, P.wo_sb[:, 2 * d:2 * d + 2, 0:512],
                    start=(d == 0), stop=(d == EC // 2 - 1),
                    perf_mode=PM.DoubleRow,
                )
                nc.tensor.matmul(
                    po[:, 512:H], lh, P.wo_sb[:, 2 * d:2 * d + 2, 512:H],
                    start=(d == 0), stop=(d == EC // 2 - 1),
                    perf_mode=PM.DoubleRow,
                )
            xr = P.tpw.tile([128, H], f32, tag="xr")
            nc.sync.dma_start(xr[:], xres[st * 128:(st + 1) * 128, :])
            y = P.yp.tile([128, H], f32, tag="y")
            ysum = P.tpw.tile([128, 1], f32, tag="ysum")
            # y-add reads PSUM -> DVE only; one 768-wide op
            nc.vector.tensor_tensor(y[:], po[:, 0:H], xr[:], OP.add)
            nc.vector.reduce_sum(ysum[:], y[:], axis=mybir.AxisListType.X)
            nc.gpsimd.tensor_scalar_mul(
                P.negmu_all[:, st:st + 1], ysum[:], -1.0 / H
            )
            sq = P.tpw.tile([128, H], f32, tag="scratch")
            nc.scalar.activation(
                sq[:], y[:], AF.Square,
                bias=P.negmu_all[:, st:st + 1], scale=1.0,
                accum_out=P.ss_all[:, st:st + 1],
            )
            ys[st] = y
        s0, s1 = sts[0], sts[-1] + 1
        nc.scalar.activation(
            P.std_all[:, s0:s1], P.ss_all[:, s0:s1], AF.Sqrt,
            bias=P.eps_ap[:, 0:1], scale=1.0 / H,
        )
        nc.vector.reciprocal(P.rstd_all[:, s0:s1], P.std_all[:, s0:s1])
        nc.vector.tensor_tensor(
            P.nmr_all[:, s0:s1], P.negmu_all[:, s0:s1],
            P.rstd_all[:, s0:s1], OP.mult,
        )
        final = sts[-1] == NSTQ - 1
        for j, st in enumerate(sts):
            # GpSimd-only mid-kernel (ACT/DVE are contended by the next qt's
            # exp stream); in the FINAL drain DVE is idle, so alternate.
            e1 = nc.vector if (final and j % 2 == 0) else nc.gpsimd
            e2 = nc.vector if (final and j % 2 == 1) else nc.gpsimd
            t2 = P.tpw.tile([128, H], f32, tag="scratch")
            e1.tensor_scalar(
                t2[:], ys[st][:],
                P.negmu_all[:, st:st + 1], P.rstd_all[:, st:st + 1],
                OP.add, OP.mult,
            )
            t3 = P.tpw.tile([128, H], f32, tag="scratch")
            e2.tensor_tensor(t3[:], t2[:], P.lng_bc[:], OP.mult)
            ot = P.tpw.tile([128, H], f32, tag="scratch")
            e1.tensor_tensor(ot[:], t3[:], P.lnb_bc[:], OP.add)
            nc.sync.dma_start(out[st * 128:(st + 1) * 128, :], ot[:])

    class P:
        pass

    with tile.TileContext(nc) as tc:
        with tc.tile_pool(name="persist", bufs=1) as pp:
            P.ctxT = pp.tile([128, EC, SQ_], f8, tag="ctxT")
            P.qb_sb = pp.tile([128, OC], f32, tag="qb")
            P.kb_sb = pp.tile([128, OC], f32, tag="kb")
            P.emsk16_sb = pp.tile([128, NST], f32, tag="em16")
            P.emsk32_sb = pp.tile([128, NST], f32, tag="em32")
            P.lng_bc = pp.tile([128, H], f32, tag="lngbc")
            P.lnb_bc = pp.tile([128, H], f32, tag="lnbbc")
            P.eps_ap = pp.tile([128, 1], f32, tag="eps")
            P.ss_all = pp.tile([128, NSTQ], f32, tag="ss")
            P.negmu_all = pp.tile([128, NSTQ], f32, tag="negmu")
            P.std_all = pp.tile([128, NSTQ], f32, tag="std")
            P.rstd_all = pp.tile([128, NSTQ], f32, tag="rstd")
            P.nmr_all = pp.tile([128, NSTQ], f32, tag="nmr")
            def persist_dmas():
                # Emitted AFTER wq+xtq on rep 0 (see emit_qkv): each small
                # DMA costs ~0.5-1.3us of queue latency, and queuing six of
                # them first delayed the PE's first matmul by ~8us.  Only
                # qb/kb are needed early (first conv); the rest queue after
                # the weight stream.
                nc.sync.dma_start(
                    P.qb_sb[:], qb8.rearrange("(c p) -> p c", p=128)
                )
                nc.sync.dma_start(
                    P.kb_sb[:], kb.rearrange("(c p) -> p c", p=128)
                )
                nc.vector.memset(P.eps_ap[:], float(LN_EPS))

            def persist_dmas_late():
                nc.sync.dma_start(
                    P.emsk16_sb[:], emsk16.rearrange("(c p) -> p c", p=128)
                )
                nc.sync.dma_start(
                    P.emsk32_sb[:], emsk32.rearrange("(c p) -> p c", p=128)
                )
                nc.sync.dma_start(
                    P.lng_bc[:], lng[None, :].to_broadcast((128, H))
                )
                nc.sync.dma_start(
                    P.lnb_bc[:], lnb[None, :].to_broadcast((128, H))
                )

            P.persist_dmas = persist_dmas
            P.persist_dmas_late = persist_dmas_late

            for rep_ in range(repeat):
                with tc.tile_pool(name="bulk", bufs=1) as bulk:
                    P.kt_sb = bulk.tile([128, OC, S_], bf16, tag="kt")
                    P.qt_sb = bulk.tile([128, OC, SQ_], bf16, tag="qt")
                    P.v_sb = bulk.tile([128, NST, (NH + 1) * (HD + 16)], f8, tag="v")
                    P.wq_sb = bulk.tile([128, EC, H], f8, tag="wq")
                    P.wk_sb = bulk.tile([128, EC, H], f8, tag="wk")
                    P.wv_sb = bulk.tile([128, EC, H], f8, tag="wv")
                    P.wo_sb = bulk.tile([128, EC, H], f8, tag="wo")
                    nc.sync.dma_start(
                        P.wq_sb[:], wq8.rearrange("(c p) o -> p c o", p=128)
                    )
                    vv_all = P.v_sb.rearrange("p t (h d) -> p t h d", d=HD + 16)
                    nc.gpsimd.memset(vv_all[:, :, :, HD + 1:HD + 16], 0.0)
                    nc.gpsimd.memset(vv_all[:, :, NH:NH + 1, :], 0.0)
                    with tc.tile_pool(name="xtp", bufs=xtb) as xtp, \
                         tc.tile_pool(name="ptp", bufs=ptb) as ptp, \
                         tc.tile_pool(name="rdp", bufs=rdb_) as rdp, \
                         tc.tile_pool(name="rddr", bufs=3, space="DRAM") as rddr, \
                         tc.tile_pool(name="tpw", bufs=3) as tpw, \
                         tc.tile_pool(name="yp", bufs=NSTQ) as yp, \
                         tc.tile_pool(name="big", bufs=spb, space="PSUM") as big, \
                         tc.tile_pool(name="psC", bufs=2, space="PSUM") as psC:
                        P.xtp, P.ptp, P.rdp, P.rddr = xtp, ptp, rdp, rddr
                        P.tpw, P.yp = tpw, yp
                        P.big, P.psC = big, psC
                        P.ecnt = [0]
                        emit_qkv(nc, P)
                        if stage == "proj":
                            nc.gpsimd.dma_start(out[0:128, :], P.qt_sb[:, :, 0:128])
                            nc.gpsimd.dma_start(out[128:256, :], P.kt_sb[:, :, 0:128])
                        else:
                            emit_attention(nc, P)
                            if stage == "attn":
                                dbgx = P.tpw.tile([128, EC, 128], f32, tag="dbgx")
                                nc.vector.tensor_copy(dbgx[:], P.ctxT[:, :, 0:128])
                                nc.gpsimd.dma_start(
                                    out[0:128, :],
                                    dbgx.rearrange("p c s -> p (c s)"),
                                )

    nc.compile()
    _BUILD_CACHE[key] = nc
    return nc


def make_in_maps(inputs, S_=S, SQ_=SQ):
    """Host-side sharding: slice/transpose/cast/scale the full inputs into
    the 8 per-core input maps."""
    f8 = ml_dtypes.float8_e4m3
    hs = np.ascontiguousarray(np.asarray(inputs["hidden_states"], np.float32))
    am = np.asarray(inputs["attention_mask"], np.float32)
    q_w = np.asarray(inputs["q_w"], np.float32)
    k_w = np.asarray(inputs["k_w"], np.float32)
    v_w = np.asarray(inputs["v_w"], np.float32)
    o_w = np.asarray(inputs["o_w"], np.float32)
    q_b = np.asarray(inputs["q_b"], np.float32)
    k_b = np.asarray(inputs["k_b"], np.float32)
    v_b = np.asarray(inputs["v_b"], np.float32)
    o_b = np.asarray(inputs["o_b"], np.float32)
    ln_g = np.asarray(inputs["ln_g"], np.float32)
    ln_b = np.asarray(inputs["ln_b"], np.float32)

    wq8_a = np.ascontiguousarray((q_w.T * 16.0).astype(f8))
    wk8_a = np.ascontiguousarray((k_w.T * 16.0).astype(f8))
    wv8_a = np.ascontiguousarray((v_w.T * 16.0).astype(f8))
    wo8_a = np.ascontiguousarray((o_w.T * 32.0).astype(f8))
    qb8_a = (q_b / 8.0).astype(np.float32)
    res_const = o_b + v_b @ o_w.T  # vb passes softmax normalization untouched

    nb = hs.shape[0]
    xT8_full = [np.ascontiguousarray(hs[b].T.astype(f8)) for b in range(nb)]
    groups = NCORES // nb  # query-parallel cores per batch

    in_maps = []
    for c in range(NCORES):
        b, j = c // groups, c % groups
        sl = slice(j * SQ_, (j + 1) * SQ_)
        em = np.exp(am[b, 0, 0]).astype(np.float32)
        in_maps.append(
            {
                "xT8": xT8_full[b],
                "xTq8": np.ascontiguousarray(xT8_full[b][:, sl]),
                "wq8": wq8_a, "wk8": wk8_a, "wv8": wv8_a, "wo8": wo8_a,
                "qb8": qb8_a, "kb": k_b,
                "emsk16": np.ascontiguousarray(em / 16.0),
                "emsk32": np.ascontiguousarray(em / 32.0),
                "xres": np.ascontiguousarray(
                    1024.0 * (hs[b, sl] + res_const[None, :])
                ),
                "lng": ln_g, "lnb": ln_b,
            }
        )
    return in_maps


def run_cores(inputs, trace=False, **kwargs):
    from concourse.bass_utils import run_bass_kernel_spmd

    nc = build()
    in_maps = make_in_maps(inputs)
    res = run_bass_kernel_spmd(
        nc, in_maps, core_ids=list(range(NCORES)), trace=trace, **kwargs
    )
    nb = np.asarray(inputs["hidden_states"]).shape[0]
    groups = NCORES // nb
    out = np.empty((nb, S, H), np.float32)
    for c in range(NCORES):
        b, j = c // groups, c % groups
        out[b, j * SQ:(j + 1) * SQ] = res.results[c]["out"]
    return out, res


def kernel(**inputs):
    out, _ = run_cores(inputs, trace=False)
    return out



# revision 72
# speedup vs baseline: 1.8425x; 1.8425x over previous
"""Trainium2 Bass kernel for a BERT-style self-attention block (B=2, S=4096,
H=768, NH=12) sharded over 8 NeuronCores.

Sharding: data-parallel over batch (2) x query-block parallel (4) = 8 cores.
Each core computes K/V for the full sequence of its batch (replicated within
the 4-core group) and a disjoint 1024-query slice of the output, so no
collectives are needed and the LayerNorm epilogue is fully local.

Per-core dataflow (fp8 DoubleRow matmuls where contraction >= 256):
  Projections: x8 [H,S] fp8, weights fp8 (scaled x16/x32 into e4m3 range),
    K^T/Q^T stored bf16 (ACT Identity conv, per-channel bias, 1/16 & 1/128
    descale), V stored fp8 with exp(mask) folded into its rows (additive
    mask == multiplicative exp(mask) on unnormalized probs) and the v-bias
    routed through softmax into the host-side residual (vb passes softmax
    normalization untouched: ctx = raw + vb).
  Scores: bf16, per head at PE row-offset (h%2)*64, true-scale in PSUM.
    Both 128-key tiles of a pair land in ONE 2-bank PSUM tile (unified
    3-buf "big" pool shared with proj/tail) so each exp op is 1024 wide.
  exp: split across ACT (AF.Exp, fp8 out) and DVE (Schraudolph: one
    tensor_scalar mult+add writing the fp8e4m3 BITS as int8), strict A/V
    alternation per skey-chunk-pair (GpSimd/DMA cannot read PSUM, so only
    these two engines can drain scores; GpSimd instead absorbs all
    SBUF-side work: LN chain, memsets, copies).
  ctx: fp8 DoubleRow over skey-chunk pairs, ones-column = exp(mask)/32 so
    row 64 of the accumulator is the softmax denominator/32; normalize =
    reciprocal_approx_fast + DRAM-bounce broadcast + one DVE mult writing
    ctxT fp8 (x32 scaled).
  Tail: O-proj fp8 DR (PSUM = 1024*ctx@o_w.T), y = po + 1024*(x+o_b+vb@o_w.T)
    (LN is scale-invariant), mean/var fp32, normalize via fused
    affine_mul_reduce (*gamma) + affine_then_add (+beta).
"""

import numpy as np
import ml_dtypes

B, S, H, NH = 2, 4096, 768, 12
HD = H // NH  # 64
NCORES = 8
SQ = S // 4  # 1024 queries per core
LN_EPS = 1e-12

# Schraudolph exp in fp8e4m3 bit-space: bits = round(s*8/ln2 + B8)
EXP_A = 8.0 / np.log(2.0)
EXP_B8 = 56.0 - 0.458

_BUILD_CACHE = {}


def build(S_=S, SQ_=SQ, stage="full", repeat=1, exp_pat="AV",
          spb=3, ptb=8, rdb_=3, xtb=3, conv_pat="VA", norm_gp=1, tail_lag=3,
          rq=False):
    key = (S_, SQ_, stage, repeat, exp_pat, spb, ptb, rdb_, xtb, conv_pat,
           norm_gp, tail_lag, rq)
    if key in _BUILD_CACHE:
        return _BUILD_CACHE[key]

    import concourse.mybir as mybir
    import concourse.tile as tile
    from concourse import bacc

    dt = mybir.dt
    f32, bf16, f8, i8 = dt.float32, dt.bfloat16, dt.float8e4, dt.int8
    AF = mybir.ActivationFunctionType
    OP = mybir.AluOpType
    PM = mybir.MatmulPerfMode

    EC = H // 128            # 6 contraction chunks over H
    OC = H // 128            # 6 output-channel chunks over H
    NSC512 = S_ // 512       # 512-wide s chunks (projection phase)
    NST = S_ // 128          # 128-wide skey tiles
    NST2 = NST // 2          # skey tile pairs (ctx DoubleRow)
    QT = min(512, SQ_)       # q tile for attention
    NQT = SQ_ // QT
    QCH = min(512, SQ_)      # q chunk for Q projection
    NQCH = SQ_ // QCH
    NSTQ = SQ_ // 128        # output s tiles

    nc = bacc.Bacc("TRN2", target_bir_lowering=False, debug=False)

    xT8 = nc.dram_tensor("xT8", [H, S_], f8, kind="ExternalInput")
    xTq8 = nc.dram_tensor("xTq8", [H, SQ_], f8, kind="ExternalInput")
    wq8 = nc.dram_tensor("wq8", [H, H], f8, kind="ExternalInput")
    wk8 = nc.dram_tensor("wk8", [H, H], f8, kind="ExternalInput")
    wv8 = nc.dram_tensor("wv8", [H, H], f8, kind="ExternalInput")
    wo8 = nc.dram_tensor("wo8", [H, H], f8, kind="ExternalInput")
    qb8 = nc.dram_tensor("qb8", [H], f32, kind="ExternalInput")
    kb = nc.dram_tensor("kb", [H], f32, kind="ExternalInput")
    emsk16 = nc.dram_tensor("emsk16", [S_], f32, kind="ExternalInput")
    emsk32 = nc.dram_tensor("emsk32", [S_], f32, kind="ExternalInput")
    xres = nc.dram_tensor("xres", [SQ_, H], f32, kind="ExternalInput")
    lng = nc.dram_tensor("lng", [H], f32, kind="ExternalInput")
    lnb = nc.dram_tensor("lnb", [H], f32, kind="ExternalInput")
    out = nc.dram_tensor("out", [SQ_, H], f32, kind="ExternalOutput")

    def emit_qkv(nc, P):
        # PSUM->SBUF conversions rotate across DVE/ACT so no single engine
        # gates the projection phase (ACT alone was 95% busy here).  GpSimd
        # cannot touch PSUM (BIR verifier), so it only gets SBUF-side work.
        cstate = {"i": 0}

        def conv(out, psum, scale, bias, af, force=None):
            if force is None:
                e = conv_pat[cstate["i"] % len(conv_pat)]
                cstate["i"] += 1
            else:
                e = force
            if e == "A":
                nc.scalar.activation(
                    out, psum, af,
                    bias=0.0 if bias is None else bias, scale=scale,
                )
            else:
                if bias is None:
                    nc.vector.tensor_scalar(out, psum, scale, None, OP.mult)
                else:
                    nc.vector.tensor_scalar(
                        out, psum, scale, bias, OP.mult, OP.add
                    )

        # Q projection first: small, unblocks attention early.  Weight DMA
        # order: wq is emitted before this (Q-proj dep); wk/wv/wo queue AFTER
        # the first xtq tile so the PE isn't stalled behind 2.4MB of weight
        # loads it doesn't need yet.
        xTq_r = xTq8.rearrange("(c p) s -> p c s", p=128)
        for qc in range(NQCH):
            xtq = P.xtp.tile([128, EC, 512], f8, tag="xt")
            nc.sync.dma_start(xtq[:, :, 0:QCH], xTq_r[:, :, qc * QCH:(qc + 1) * QCH])
            if qc == 0:
                if P.persist_dmas is not None:
                    P.persist_dmas()
                    P.persist_dmas = None
                for wsb, wdr in ((P.wk_sb, wk8), (P.wv_sb, wv8),
                                 (P.wo_sb, wo8)):
                    nc.sync.dma_start(
                        wsb[:], wdr.rearrange("(c p) o -> p c o", p=128)
                    )
                if P.persist_dmas_late is not None:
                    P.persist_dmas_late()
                    P.persist_dmas_late = None
            for oc2 in range(OC // 2):
                pq = P.big.tile([128, 2 * 512], f32, tag="big")
                for half in (0, 1):
                    oc = 2 * oc2 + half
                    for e in range(EC // 2):
                        nc.tensor.matmul(
                            pq[:, half * 512:half * 512 + QCH],
                            P.wq_sb[:, 2 * e:2 * e + 2, oc * 128:(oc + 1) * 128],
                            xtq[:, 2 * e:2 * e + 2, 0:QCH],
                            start=(e == 0), stop=(e == EC // 2 - 1),
                            perf_mode=PM.DoubleRow,
                        )
                for half in (0, 1):
                    oc = 2 * oc2 + half
                    conv(
                        P.qt_sb[:, oc, qc * QCH:(qc + 1) * QCH],
                        pq[:, half * 512:half * 512 + QCH],
                        1.0 / 128.0, P.qb_sb[:, oc:oc + 1], AF.Identity,
                    )
        # K^T and V over the full sequence
        xT_r = xT8.rearrange("(c p) s -> p c s", p=128)
        for sc in range(NSC512):
            xt = P.xtp.tile([128, EC, 512], f8, tag="xt")
            nc.sync.dma_start(xt[:], xT_r[:, :, sc * 512:(sc + 1) * 512])
            for oc2 in range(OC // 2):
                pk = P.big.tile([128, 2 * 512], f32, tag="big")
                for half in (0, 1):
                    oc = 2 * oc2 + half
                    for e in range(EC // 2):
                        nc.tensor.matmul(
                            pk[:, half * 512:(half + 1) * 512],
                            P.wk_sb[:, 2 * e:2 * e + 2, oc * 128:(oc + 1) * 128],
                            xt[:, 2 * e:2 * e + 2, :],
                            start=(e == 0), stop=(e == EC // 2 - 1),
                            perf_mode=PM.DoubleRow,
                        )
                for half in (0, 1):
                    oc = 2 * oc2 + half
                    conv(
                        P.kt_sb[:, oc, sc * 512:(sc + 1) * 512],
                        pk[:, half * 512:(half + 1) * 512],
                        1.0 / 16.0, P.kb_sb[:, oc:oc + 1], AF.Identity,
                    )
            for t4 in range(4):
                st = sc * 4 + t4
                # One 2-bank tile holds both V-proj halves contiguously so a
                # single conv op covers all 768 channels.
                pv = P.big.tile([128, 2 * QT], f32, tag="big")
                for e in range(EC // 2):
                    xs = xt[:, 2 * e:2 * e + 2, t4 * 128:(t4 + 1) * 128]
                    nc.tensor.matmul(
                        pv[:, 0:512], xs, P.wv_sb[:, 2 * e:2 * e + 2, 0:512],
                        start=(e == 0), stop=(e == EC // 2 - 1),
                        perf_mode=PM.DoubleRow,
                    )
                    nc.tensor.matmul(
                        pv[:, 512:H], xs, P.wv_sb[:, 2 * e:2 * e + 2, 512:H],
                        start=(e == 0), stop=(e == EC // 2 - 1),
                        perf_mode=PM.DoubleRow,
                    )
                vview = P.v_sb[:, st].rearrange("p (h d) -> p h d", d=HD + 16)
                conv(
                    vview[:, 0:NH, 0:HD],
                    pv[:, 0:H].rearrange("p (h d) -> p h d", d=HD),
                    P.emsk16_sb[:, st:st + 1], None, AF.Copy,
                )
                nc.gpsimd.tensor_copy(
                    vview[:, 0:NH, HD:HD + 1],
                    P.emsk32_sb[:, st:st + 1].to_broadcast((128, NH, 1)),
                )

    def emit_attention(nc, P):
        do_norm = stage not in ("attn1",)
        pend = []

        def flush_one():
            # consume one queued score tile: exp + ctx-DR (+ normalize at
            # the head's last pair)
            qt, h, sc2, cx, sp, pt = pend.pop(0)
            hp = (h % 2) * 64
            oc = h // 2
            q0 = qt * QT
            pt_flat = pt.rearrange("p a q -> p (a q)")
            eng = exp_pat[P.ecnt[0] % len(exp_pat)]
            P.ecnt[0] += 1
            if eng == "A":
                nc.scalar.activation(
                    pt_flat, sp[:], AF.Exp, bias=0.0, scale=1.0
                )
            else:
                nc.vector.tensor_scalar(
                    pt_flat.bitcast(i8), sp[:],
                    EXP_A, EXP_B8, OP.mult, OP.add,
                )
            nc.tensor.matmul(
                cx[:],
                P.v_sb[:, 2 * sc2:2 * sc2 + 2,
                       h * (HD + 16):h * (HD + 16) + 128],
                pt[:],
                start=(sc2 == 0), stop=(sc2 == NST2 - 1),
                perf_mode=PM.DoubleRow,
            )
            if sc2 == NST2 - 1:
                emit_norm(qt, h, cx)

        def emit_norm(qt, h, cx):
            hp = (h % 2) * 64
            oc = h // 2
            q0 = qt * QT
            if do_norm:
                    if stage == "attn" and qt == 0 and h == 0:
                        den = P.rdp.tile([1, QT], f32, tag="den")
                        nc.vector.tensor_copy(den[:], cx[HD:HD + 1, :])
                        nc.gpsimd.dma_start(out[128:129, 0:QT], den[:])
                    if norm_gp and h % norm_gp == 0:
                        # Drain cx (plus the denominator row) via one ACT
                        # copy so the (SBUF-only, mostly idle) GpSimd can do
                        # the normalize multiply; walrus rejects divide on
                        # Pool, so DVE still does the reciprocal — but on a
                        # [64,8] DRAM-bounced view (DVE cost is free-size
                        # driven: 8 cols, not 512).
                        if rq:
                            # cxs copy alternates A/V; the den bounce rides
                            # the Pool DMA queue (sync stays clear) and the
                            # [64,8] view makes the DVE reciprocal ~free.
                            cxs = P.rdp.tile([HD + 1, QT], f32, tag="cxs")
                            if h % 2 == 0:
                                nc.scalar.activation(
                                    cxs[:], cx[0:HD + 1, :], AF.Copy,
                                    bias=0.0, scale=1.0,
                                )
                            else:
                                nc.vector.tensor_scalar(
                                    cxs[:], cx[0:HD + 1, :], 1.0, None,
                                    OP.mult,
                                )
                            rdd = P.rddr.tile([1, QT], f32, tag="rdd")
                            nc.gpsimd.dma_start(rdd[:], cxs[HD:HD + 1, :])
                            rds = P.rdp.tile([64, QT // 64], f32, tag="rds")
                            nc.gpsimd.dma_start(
                                rds[:],
                                rdd[:].rearrange("o (p c) -> (o p) c", p=64),
                            )
                            rdi = P.rdp.tile([64, QT // 64], f32, tag="rdi")
                            nc.vector.reciprocal(rdi[:], rds[:])
                            rdd2 = P.rddr.tile([64, QT // 64], f32, tag="rdd2")
                            nc.gpsimd.dma_start(rdd2[:], rdi[:])
                            rdb = P.rdp.tile([64, QT], f32, tag="rdb")
                            nc.gpsimd.dma_start(
                                rdb[:],
                                rdd2[:].rearrange("(o p) c -> o (p c)", o=1)
                                .to_broadcast((64, QT)),
                            )
                        else:
                            # recip first: it heads the longest chain
                            # (recip -> DMA -> DMA -> mult)
                            rd = P.rdp.tile([1, QT], f32, tag="rd")
                            nc.vector.reciprocal(rd[:], cx[HD:HD + 1, :])
                            cxs = P.rdp.tile([HD, QT], f32, tag="cxs")
                            nc.scalar.activation(
                                cxs[:], cx[0:HD, :], AF.Copy,
                                bias=0.0, scale=1.0,
                            )
                            rdd = P.rddr.tile([1, QT], f32, tag="rdd")
                            nc.sync.dma_start(rdd[:], rd[:])
                            rdb = P.rdp.tile([64, QT], f32, tag="rdb")
                            nc.sync.dma_start(
                                rdb[:], rdd[:].to_broadcast((64, QT))
                            )
                        nc.gpsimd.tensor_tensor(
                            P.ctxT[hp:hp + 64, oc, q0:q0 + QT],
                            cxs[0:HD, :], rdb[:], OP.mult,
                        )
                    else:
                        rd = P.rdp.tile([1, QT], f32, tag="rd")
                        nc.vector.reciprocal(rd[:], cx[HD:HD + 1, :])
                        rdd = P.rddr.tile([1, QT], f32, tag="rdd")
                        nc.sync.dma_start(rdd[:], rd[:])
                        rdb = P.rdp.tile([64, QT], f32, tag="rdb")
                        nc.sync.dma_start(rdb[:], rdd[:].to_broadcast((64, QT)))
                        nc.vector.tensor_tensor(
                            P.ctxT[hp:hp + 64, oc, q0:q0 + QT],
                            cx[0:HD, :], rdb[:], OP.mult,
                        )

        # Driver: score MMs lead exp/ctx consumption by `plag` tiles, with
        # the pipeline carried ACROSS head boundaries — the next head's
        # first tile fills while the previous head's last exp drains, which
        # removes the ~780ns per-head DVE stall the gap profile showed.
        plag = 2
        for qt in range(NQT):
            q0 = qt * QT
            for h in range(NH):
                if stage == "full" and qt > 0 and h == tail_lag:
                    while pend:
                        flush_one()
                    emit_tail(nc, P, list(range((qt - 1) * (QT // 128),
                                                qt * (QT // 128))))
                hp = (h % 2) * 64
                oc = h // 2
                cx = P.psC.tile([128, QT], f32, tag="cx")
                for sc2 in range(NST2):
                    pt = P.ptp.tile([128, 2, QT], f8, tag="pt")
                    # Both 128-key score tiles land in ONE 2-bank PSUM tile
                    # so a single 1024-wide exp op covers the pair.
                    sp = P.big.tile([128, 2 * QT], f32, tag="big")
                    for par in (0, 1):
                        sc = 2 * sc2 + par
                        nc.tensor.matmul(
                            sp[:, par * QT:(par + 1) * QT],
                            P.kt_sb[hp:hp + 64, oc, sc * 128:(sc + 1) * 128],
                            P.qt_sb[hp:hp + 64, oc, q0:q0 + QT],
                            start=True, stop=True,
                        )
                    pend.append((qt, h, sc2, cx, sp, pt))
                    while len(pend) > plag:
                        flush_one()
            if stage == "full" and (qt == NQT - 1 or tail_lag == 0):
                while pend:
                    flush_one()
                emit_tail(nc, P, list(range(qt * (QT // 128),
                                            (qt + 1) * (QT // 128))))
        while pend:
            flush_one()

    def emit_tail(nc, P, sts):
        ys = {}
        for st in sts:
            # Both O-proj halves in one 2-bank tile: the y-add below becomes
            # a single 768-wide op.
            po = P.big.tile([128, 2 * QT], f32, tag="big")
            for d in range(EC // 2):
                lh = P.ctxT[:, 2 * d:2 * d + 2, st * 128:(st + 1) * 128]
                nc.tensor.matmul(
                    po[:, 0:512], lh, P.wo_sb[:, 2 * d:2 * d + 2, 0:512],
                    start=(d == 0), stop=(d == EC // 2 - 1),
                    perf_mode=PM.DoubleRow,
                )
                nc.tensor.matmul(
                    po[:, 512:H], lh, P.wo_sb[:, 2 * d:2 * d + 2, 512:H],
                    start=(d == 0), stop=(d == EC // 2 - 1),
                    perf_mode=PM.DoubleRow,
                )
            xr = P.tpw.tile([128, H], f32, tag="xr")
            nc.sync.dma_start(xr[:], xres[st * 128:(st + 1) * 128, :])
            y = P.yp.tile([128, H], f32, tag="y")
            ysum = P.tpw.tile([128, 1], f32, tag="ysum")
            # y-add reads PSUM -> DVE only; one 768-wide op
            nc.vector.tensor_tensor(y[:], po[:, 0:H], xr[:], OP.add)
            nc.vector.reduce_sum(ysum[:], y[:], axis=mybir.AxisListType.X)
            nc.gpsimd.tensor_scalar_mul(
                P.negmu_all[:, st:st + 1], ysum[:], -1.0 / H
            )
            sq = P.tpw.tile([128, H], f32, tag="scratch")
            nc.scalar.activation(
                sq[:], y[:], AF.Square,
                bias=P.negmu_all[:, st:st + 1], scale=1.0,
                accum_out=P.ss_all[:, st:st + 1],
            )
            ys[st] = y
        s0, s1 = sts[0], sts[-1] + 1
        nc.scalar.activation(
            P.std_all[:, s0:s1], P.ss_all[:, s0:s1], AF.Sqrt,
            bias=P.eps_ap[:, 0:1], scale=1.0 / H,
        )
        nc.vector.reciprocal(P.rstd_all[:, s0:s1], P.std_all[:, s0:s1])
        nc.vector.tensor_tensor(
            P.nmr_all[:, s0:s1], P.negmu_all[:, s0:s1],
            P.rstd_all[:, s0:s1], OP.mult,
        )
        final = sts[-1] == NSTQ - 1
        for j, st in enumerate(sts):
            # GpSimd-only mid-kernel (ACT/DVE are contended by the next qt's
            # exp stream); in the FINAL drain DVE is idle, so alternate.
            e1 = nc.vector if (final and j % 2 == 0) else nc.gpsimd
            e2 = nc.vector if (final and j % 2 == 1) else nc.gpsimd
            t2 = P.tpw.tile([128, H], f32, tag="scratch")
            e1.tensor_scalar(
                t2[:], ys[st][:],
                P.negmu_all[:, st:st + 1], P.rstd_all[:, st:st + 1],
                OP.add, OP.mult,
            )
            t3 = P.tpw.tile([128, H], f32, tag="scratch")
            e2.tensor_tensor(t3[:], t2[:], P.lng_bc[:], OP.mult)
            ot = P.tpw.tile([128, H], f32, tag="scratch")
            e1.tensor_tensor(ot[:], t3[:], P.lnb_bc[:], OP.add)
            nc.sync.dma_start(out[st * 128:(st + 1) * 128, :], ot[:])

    class P:
        pass

    with tile.TileContext(nc) as tc:
        with tc.tile_pool(name="persist", bufs=1) as pp:
            P.ctxT = pp.tile([128, EC, SQ_], f8, tag="ctxT")
            P.qb_sb = pp.tile([128, OC], f32, tag="qb")
            P.kb_sb = pp.tile([128, OC], f32, tag="kb")
            P.emsk16_sb = pp.tile([128, NST], f32, tag="em16")
            P.emsk32_sb = pp.tile([128, NST], f32, tag="em32")
            P.lng_bc = pp.tile([128, H], f32, tag="lngbc")
            P.lnb_bc = pp.tile([128, H], f32, tag="lnbbc")
            P.eps_ap = pp.tile([128, 1], f32, tag="eps")
            P.ss_all = pp.tile([128, NSTQ], f32, tag="ss")
            P.negmu_all = pp.tile([128, NSTQ], f32, tag="negmu")
            P.std_all = pp.tile([128, NSTQ], f32, tag="std")
            P.rstd_all = pp.tile([128, NSTQ], f32, tag="rstd")
            P.nmr_all = pp.tile([128, NSTQ], f32, tag="nmr")
            def persist_dmas():
                # Emitted AFTER wq+xtq on rep 0 (see emit_qkv): each small
                # DMA costs ~0.5-1.3us of queue latency, and queuing six of
                # them first delayed the PE's first matmul by ~8us.  Only
                # qb/kb are needed early (first conv); the rest queue after
                # the weight stream.
                nc.sync.dma_start(
                    P.qb_sb[:], qb8.rearrange("(c p) -> p c", p=128)
                )
                nc.sync.dma_start(
                    P.kb_sb[:], kb.rearrange("(c p) -> p c", p=128)
                )
                nc.vector.memset(P.eps_ap[:], float(LN_EPS))

            def persist_dmas_late():
                nc.sync.dma_start(
                    P.emsk16_sb[:], emsk16.rearrange("(c p) -> p c", p=128)
                )
                nc.sync.dma_start(
                    P.emsk32_sb[:], emsk32.rearrange("(c p) -> p c", p=128)
                )
                nc.sync.dma_start(
                    P.lng_bc[:], lng[None, :].to_broadcast((128, H))
                )
                nc.sync.dma_start(
                    P.lnb_bc[:], lnb[None, :].to_broadcast((128, H))
                )

            P.persist_dmas = persist_dmas
            P.persist_dmas_late = persist_dmas_late

            for rep_ in range(repeat):
                with tc.tile_pool(name="bulk", bufs=1) as bulk:
                    P.kt_sb = bulk.tile([128, OC, S_], bf16, tag="kt")
                    P.qt_sb = bulk.tile([128, OC, SQ_], bf16, tag="qt")
                    P.v_sb = bulk.tile([128, NST, (NH + 1) * (HD + 16)], f8, tag="v")
                    P.wq_sb = bulk.tile([128, EC, H], f8, tag="wq")
                    P.wk_sb = bulk.tile([128, EC, H], f8, tag="wk")
                    P.wv_sb = bulk.tile([128, EC, H], f8, tag="wv")
                    P.wo_sb = bulk.tile([128, EC, H], f8, tag="wo")
                    nc.sync.dma_start(
                        P.wq_sb[:], wq8.rearrange("(c p) o -> p c o", p=128)
                    )
                    vv_all = P.v_sb.rearrange("p t (h d) -> p t h d", d=HD + 16)
                    nc.gpsimd.memset(vv_all[:, :, :, HD + 1:HD + 16], 0.0)
                    nc.gpsimd.memset(vv_all[:, :, NH:NH + 1, :], 0.0)
                    with tc.tile_pool(name="xtp", bufs=xtb) as xtp, \
                         tc.tile_pool(name="ptp", bufs=ptb) as ptp, \
                         tc.tile_pool(name="rdp", bufs=rdb_) as rdp, \
                         tc.tile_pool(name="rddr", bufs=3, space="DRAM") as rddr, \
                         tc.tile_pool(name="tpw", bufs=3) as tpw, \
                         tc.tile_pool(name="yp", bufs=NSTQ) as yp, \
                         tc.tile_pool(name="big", bufs=spb, space="PSUM") as big, \
                         tc.tile_pool(name="psC", bufs=2, space="PSUM") as psC:
                        P.xtp, P.ptp, P.rdp, P.rddr = xtp, ptp, rdp, rddr
                        P.tpw, P.yp = tpw, yp
                        P.big, P.psC = big, psC
                        P.ecnt = [0]
                        emit_qkv(nc, P)
                        if stage == "proj":
                            nc.gpsimd.dma_start(out[0:128, :], P.qt_sb[:, :, 0:128])
                            nc.gpsimd.dma_start(out[128:256, :], P.kt_sb[:, :, 0:128])
                        else:
                            emit_attention(nc, P)
                            if stage == "attn":
                                dbgx = P.tpw.tile([128, EC, 128], f32, tag="dbgx")
                                nc.vector.tensor_copy(dbgx[:], P.ctxT[:, :, 0:128])
                                nc.gpsimd.dma_start(
                                    out[0:128, :],
                                    dbgx.rearrange("p c s -> p (c s)"),
                                )

    nc.compile()
    _BUILD_CACHE[key] = nc
    return nc


def make_in_maps(inputs, S_=S, SQ_=SQ):
    """Host-side sharding: slice/transpose/cast/scale the full inputs into
    the 8 per-core input maps."""
    f8 = ml_dtypes.float8_e4m3
    hs = np.ascontiguousarray(np.asarray(inputs["hidden_states"], np.float32))
    am = np.asarray(inputs["attention_mask"], np.float32)
    q_w = np.asarray(inputs["q_w"], np.float32)
    k_w = np.asarray(inputs["k_w"], np.float32)
    v_w = np.asarray(inputs["v_w"], np.float32)
    o_w = np.asarray(inputs["o_w"], np.float32)
    q_b = np.asarray(inputs["q_b"], np.float32)
    k_b = np.asarray(inputs["k_b"], np.float32)
    v_b = np.asarray(inputs["v_b"], np.float32)
    o_b = np.asarray(inputs["o_b"], np.float32)
    ln_g = np.asarray(inputs["ln_g"], np.float32)
    ln_b = np.asarray(inputs["ln_b"], np.float32)

    wq8_a = np.ascontiguousarray((q_w.T * 16.0).astype(f8))
    wk8_a = np.ascontiguousarray((k_w.T * 16.0).astype(f8))
    wv8_a = np.ascontiguousarray((v_w.T * 16.0).astype(f8))
    wo8_a = np.ascontiguousarray((o_w.T * 32.0).astype(f8))
    qb8_a = (q_b / 8.0).astype(np.float32)
    res_const = o_b + v_b @ o_w.T  # vb passes softmax normalization untouched

    nb = hs.shape[0]
    xT8_full = [np.ascontiguousarray(hs[b].T.astype(f8)) for b in range(nb)]
    groups = NCORES // nb  # query-parallel cores per batch

    in_maps = []
    for c in range(NCORES):
        b, j = c // groups, c % groups
        sl = slice(j * SQ_, (j + 1) * SQ_)
        em = np.exp(am[b, 0, 0]).astype(np.float32)
        in_maps.append(
            {
                "xT8": xT8_full[b],
                "xTq8": np.ascontiguousarray(xT8_full[b][:, sl]),
                "wq8": wq8_a, "wk8": wk8_a, "wv8": wv8_a, "wo8": wo8_a,
                "qb8": qb8_a, "kb": k_b,
                "emsk16": np.ascontiguousarray(em / 16.0),
                "emsk32": np.ascontiguousarray(em / 32.0),
                "xres": np.ascontiguousarray(
                    1024.0 * (hs[b, sl] + res_const[None, :])
                ),
                "lng": ln_g, "lnb": ln_b,
            }
        )
    return in_maps


def run_cores(inputs, trace=False, **kwargs):
    from concourse.bass_utils import run_bass_kernel_spmd

    nc = build()
    in_maps = make_in_maps(inputs)
    res = run_bass_kernel_spmd(
        nc, in_maps, core_ids=list(range(NCORES)), trace=trace, **kwargs
    )
    nb = np.asarray(inputs["hidden_states"]).shape[0]
    groups = NCORES // nb
    out = np.empty((nb, S, H), np.float32)
    for c in range(NCORES):
        b, j = c // groups, c % groups
        out[b, j * SQ:(j + 1) * SQ] = res.results[c]["out"]
    return out, res


def kernel(**inputs):
    out, _ = run_cores(inputs, trace=False)
    return out

